# revision 1
# baseline (speedup 1.0000x reference)
"""Gaussian falloff vortex-velocity kernel for Trainium2 (8 NeuronCores).

Math: out[b,h,w,:] = sum_n tau_n * exp(-r2/sig_n^2) / sqrt(r2) * (d2, -d1)
with d1 = py - y_n, d2 = px - x_n, r2 = d1^2 + d2^2.

Key identities used on device:
  exp(-r2/sig^2)/sqrt(r2) = exp(nisg*(r2 + c*ln(r2))),  nisg=-1/sig^2, c=0.5*sig^2
  out_u = px*S0 - S1,  out_v = S2 - py*S0
  where S0 = sum tau*g, S1 = sum tau*x*g, S2 = sum tau*y*g  (3-col matmul over n)

Sharding: grid rows H are split across the 8 cores (32 rows each); every core
streams all 512 particles (replicated params).
"""

import sys

import numpy as np

B, H, W, N = 2, 256, 256, 512
NCORES = 8
HPC = H // NCORES          # 32 rows per core
PPB = HPC * W              # 8192 points per batch per core
NT = PPB // 512            # 16 point-tiles of 512 per batch
NK = N // 128              # 4 particle blocks

_cache = {}


def _bass_modules():
    if "/opt/trn_rl_repo" not in sys.path:
        sys.path.insert(0, "/opt/trn_rl_repo")
    import concourse.bass as bass
    import concourse.mybir as mybir
    import concourse.tile as tile
    from concourse import bacc
    from concourse.bass_utils import run_bass_kernel_spmd

    return bass, mybir, tile, run_bass_kernel_spmd, bacc


def _build_nc():
    bass, mybir, tile, _, bacc = _bass_modules()
    f32 = mybir.dt.float32
    AF = mybir.ActivationFunctionType
    ALU = mybir.AluOpType

    nc = bacc.Bacc(None)
    params_d = nc.declare_dram_parameter("params", [128, 32], f32, isOutput=False)
    wmat_d = nc.declare_dram_parameter("wmat", [128, 24], f32, isOutput=False)
    ptsb_d = nc.declare_dram_parameter("ptsb", [B, NT, 2, 128, 512], f32, isOutput=False)
    ptsf_d = nc.declare_dram_parameter("ptsf", [B, 2, 128, PPB // 128], f32, isOutput=False)
    out_d = nc.declare_dram_parameter("out", [B, 2, 128, PPB // 128], f32, isOutput=True)

    with tile.TileContext(nc) as tc:
        with (
            tc.tile_pool(name="const", bufs=1) as cpool,
            tc.tile_pool(name="temps", bufs=3) as temps,
            tc.tile_pool(name="psbc", bufs=2) as psbc,
            tc.tile_pool(name="psacc", bufs=2, space=bass.MemorySpace.PSUM) as psacc,
            tc.tile_pool(name="fin", bufs=2) as fin,
            tc.tile_pool(name="dscratch", bufs=1, space="DRAM") as dpool,
        ):
            params = cpool.tile([128, 32], f32)
            nc.sync.dma_start(params[:], params_d[:])
            wmat = cpool.tile([128, 24], f32)
            nc.sync.dma_start(wmat[:], wmat_d[:])
            scratch = dpool.tile([B, 3, PPB], f32)

            for b in range(B):
                for T in range(NT):
                    sl = slice(T * 512, (T + 1) * 512)
                    pyb = psbc.tile([128, 512], f32, tag="pyb")
                    nc.sync.dma_start(pyb[:], ptsb_d[b, T, 0])
                    pxb = psbc.tile([128, 512], f32, tag="pxb")
                    nc.sync.dma_start(pxb[:], ptsb_d[b, T, 1])
                    sacc = psacc.tile([3, 512], f32, tag="sacc")
                    for k in range(NK):
                        c = b * NK + k
                        d1sq = temps.tile([128, 512], f32, tag="d1sq")
                        nc.scalar.activation(
                            d1sq[:], pyb[:], AF.Square, bias=params[:, c : c + 1]
                        )
                        d2 = temps.tile([128, 512], f32, tag="d2")
                        nc.vector.tensor_scalar_add(
                            d2[:], pxb[:], params[:, 8 + c : 9 + c]
                        )
                        d2sq = temps.tile([128, 512], f32, tag="d2sq")
                        nc.gpsimd.tensor_mul(d2sq[:], d2[:], d2[:])
                        r2 = temps.tile([128, 512], f32, tag="r2")
                        nc.vector.tensor_add(r2[:], d1sq[:], d2sq[:])
                        lt = temps.tile([128, 512], f32, tag="lt")
                        nc.scalar.activation(lt[:], r2[:], AF.Ln)
                        wt = temps.tile([128, 512], f32, tag="wt")
                        nc.vector.scalar_tensor_tensor(
                            wt[:], lt[:], params[:, 24 + c : 25 + c], r2[:],
                            ALU.mult, ALU.add,
                        )
                        g = temps.tile([128, 512], f32, tag="g")
                        nc.scalar.activation(
                            g[:], wt[:], AF.Exp, scale=params[:, 16 + c : 17 + c]
                        )
                        nc.tensor.matmul(
                            sacc[:], wmat[:, c * 3 : (c + 1) * 3], g[:],
                            start=(k == 0), stop=(k == NK - 1),
                        )
                    srow = temps.tile([3, 512], f32, tag="srow")
                    nc.scalar.copy(srow[:], sacc[:])
                    nc.sync.dma_start(scratch[b, :, sl], srow[:])

            srs = scratch[:].rearrange("b three (p f) -> b three p f", p=128)
            for b in range(B):
                s0 = fin.tile([128, PPB // 128], f32, tag="s0")
                nc.sync.dma_start(s0[:], srs[b, 0])
                s1 = fin.tile([128, PPB // 128], f32, tag="s1")
                nc.sync.dma_start(s1[:], srs[b, 1])
                s2 = fin.tile([128, PPB // 128], f32, tag="s2")
                nc.sync.dma_start(s2[:], srs[b, 2])
                pyf = fin.tile([128, PPB // 128], f32, tag="pyf")
                nc.sync.dma_start(pyf[:], ptsf_d[b, 0])
                pxf = fin.tile([128, PPB // 128], f32, tag="pxf")
                nc.sync.dma_start(pxf[:], ptsf_d[b, 1])
                tu = fin.tile([128, PPB // 128], f32, tag="tu")
                nc.vector.tensor_mul(tu[:], pxf[:], s0[:])
                u = fin.tile([128, PPB // 128], f32, tag="u")
                nc.vector.tensor_sub(u[:], tu[:], s1[:])
                tv = fin.tile([128, PPB // 128], f32, tag="tv")
                nc.vector.tensor_mul(tv[:], pyf[:], s0[:])
                v = fin.tile([128, PPB // 128], f32, tag="v")
                nc.vector.tensor_sub(v[:], s2[:], tv[:])
                nc.sync.dma_start(out_d[b, 0], u[:])
                nc.sync.dma_start(out_d[b, 1], v[:])
    nc.compile()
    return nc


def _prep_inputs(vortex_feature, points):
    vf = np.asarray(vortex_feature, dtype=np.float32)
    pts_full = np.asarray(points, dtype=np.float32)

    y = vf[:, :, 0]
    x = vf[:, :, 1]
    tau = vf[:, :, 2]
    sig = vf[:, :, 3]
    sig2 = sig * sig
    nisg = -1.0 / sig2
    chalf = 0.5 * sig2

    def blk(a):  # [B, N] -> [128, B*NK] with col = b*NK+k
        return np.ascontiguousarray(
            a.reshape(B, NK, 128).transpose(2, 0, 1).reshape(128, B * NK)
        )

    params = np.zeros((128, 32), dtype=np.float32)
    params[:, 0:8] = blk(-y)
    params[:, 8:16] = blk(-x)
    params[:, 16:24] = blk(nisg)
    params[:, 24:32] = blk(chalf)

    wfull = np.stack([tau, tau * x, tau * y], axis=-1)  # [B, N, 3]
    wmat = np.ascontiguousarray(
        wfull.reshape(B, NK, 128, 3).transpose(2, 0, 1, 3).reshape(128, B * NK * 3)
    )

    in_maps = []
    for i in range(NCORES):
        sl = pts_full[:, i * HPC : (i + 1) * HPC]          # [B, 32, 256, 2]
        flat = sl.reshape(B, PPB, 2)
        pts = np.ascontiguousarray(flat.transpose(0, 2, 1))  # [B, 2, PPB]
        ptsf = np.ascontiguousarray(pts.reshape(B, 2, 128, PPB // 128))
        # pre-broadcast rows: [B, NT, 2, 128, 512]
        ptsb = np.ascontiguousarray(
            np.broadcast_to(
                pts.reshape(B, 2, NT, 1, 512).transpose(0, 2, 1, 3, 4),
                (B, NT, 2, 128, 512),
            )
        )
        in_maps.append({"params": params, "wmat": wmat, "ptsb": ptsb, "ptsf": ptsf})
    return in_maps


def _assemble(results):
    out = np.zeros((B, H, W, 2), dtype=np.float32)
    for i in range(NCORES):
        o = np.asarray(results[i]["out"])  # [B, 2, 128, PPB//128]
        o = o.reshape(B, 2, PPB).transpose(0, 2, 1).reshape(B, HPC, W, 2)
        out[:, i * HPC : (i + 1) * HPC] = o
    return out


def _run(vortex_feature, points, trace=False):
    _, _, _, run_bass_kernel_spmd, _b = _bass_modules()
    if "nc" not in _cache:
        _cache["nc"] = _build_nc()
    in_maps = _prep_inputs(vortex_feature, points)
    res = run_bass_kernel_spmd(
        _cache["nc"], in_maps, list(range(NCORES)), trace=trace
    )
    return _assemble(res.results), res


def kernel(vortex_feature, points):
    out, _ = _run(vortex_feature, points, trace=False)
    return out



# revision 5
# speedup vs baseline: 1.3110x; 1.3110x over previous
"""Gaussian falloff vortex-velocity kernel for Trainium2 (8 NeuronCores).

Math: out[b,h,w,:] = sum_n tau_n * exp(-r2/sig_n^2) / sqrt(r2) * (d2, -d1)
with d1 = py - y_n, d2 = px - x_n, r2 = d1^2 + d2^2.

Device-side structure (per core, H split 8 ways):
  1. r2 via TensorE:  r2 = pp - 2y*py - 2x*px + (y^2+x^2+eps), expanded as an
     8-row fp16 matmul (hi/lo split of each operand keeps fp32-level accuracy;
     products are exact in fp16xfp16->fp32, accumulation is fp32 PSUM).
     Output tile: [128 particles, 1024 points] across 2 PSUM banks.
  2. ACT:  lt = Ln(r2_mm + vv)         (bias = vv = y^2+x^2+eps, per-partition)
     DVE:  wt = chalf * lt + r2_mm     (chalf = 0.5*sig^2)
     ACT:  g  = Exp(nisg * wt + nisg*vv) -> fp16   (nisg = -1/sig^2)
     which equals exp(-r2/sig^2)/sqrt(r2).
  3. S-sums via TensorE: [6,512] = [tau,tau*x,tau*y] (hi/lo fp16) contracted
     over 128 particles, accumulated over the 4 particle blocks in PSUM,
     partition-stacked at offsets {0,32} for the two point-tiles of a chunk.
  4. u = px*S0 - S1, v = S2 - py*S0 on DVE.
"""

import sys

import numpy as np

B, H, W, N = 2, 256, 256, 512
NCORES = 8
HPC = H // NCORES          # 32 rows per core
PPB = HPC * W              # 8192 points per batch per core
NPT = PPB // 512           # 16 point-tiles of 512 per batch
NK = N // 128              # 4 particle blocks
EPS = 4e-6                 # keeps matmul-expanded r2 strictly positive

_cache = {}


def _bass_modules():
    if "/opt/trn_rl_repo" not in sys.path:
        sys.path.insert(0, "/opt/trn_rl_repo")
    import concourse.bass as bass
    import concourse.mybir as mybir
    import concourse.tile as tile
    from concourse import bacc
    from concourse.bass_utils import run_bass_kernel_spmd

    return bass, mybir, tile, run_bass_kernel_spmd, bacc


def _build_nc():
    bass, mybir, tile, _, bacc = _bass_modules()
    f32 = mybir.dt.float32
    f16 = mybir.dt.float16
    AF = mybir.ActivationFunctionType
    ALU = mybir.AluOpType

    nc = bacc.Bacc(None)
    params_d = nc.declare_dram_parameter("params", [128, 32], f32, isOutput=False)
    wr2_d = nc.declare_dram_parameter("wr2", [8, B * NK * 128], f16, isOutput=False)
    wg_d = nc.declare_dram_parameter("wg", [128, B * NK * 6], f16, isOutput=False)
    xrows_d = nc.declare_dram_parameter("xrows", [B, 8, PPB], f16, isOutput=False)
    ptsf_d = nc.declare_dram_parameter("ptsf", [B, 2, 128, 64], f32, isOutput=False)
    out_d = nc.declare_dram_parameter("out", [B, 2, 128, 64], f32, isOutput=True)

    with tile.TileContext(nc) as tc:
        with (
            tc.tile_pool(name="const", bufs=1) as cpool,
            tc.tile_pool(name="xrows", bufs=2) as xpool,
            tc.tile_pool(name="lt", bufs=3) as ltp,
            tc.tile_pool(name="wt", bufs=3) as wtp,
            tc.tile_pool(name="g", bufs=3) as gp,
            tc.tile_pool(name="stage", bufs=2) as stp,
            tc.tile_pool(name="sd", bufs=2) as sdp,
            tc.tile_pool(name="fin", bufs=2) as finp,
            tc.tile_pool(name="psr2", bufs=2, space=bass.MemorySpace.PSUM) as psr2,
            tc.tile_pool(name="psac", bufs=2, space=bass.MemorySpace.PSUM) as psac,
        ):
            params = cpool.tile([128, 32], f32)
            nc.sync.dma_start(params[:], params_d[:])
            wr2s = cpool.tile([8, B * NK * 128], f16)
            nc.sync.dma_start(wr2s[:], wr2_d[:])
            wgs = cpool.tile([128, B * NK * 6], f16)
            nc.sync.dma_start(wgs[:], wg_d[:])
            pf = cpool.tile([128, B * 2 * 64], f32)
            for b in range(B):
                for a in range(2):
                    nc.sync.dma_start(
                        pf[:, (b * 2 + a) * 64 : (b * 2 + a + 1) * 64],
                        ptsf_d[b, a],
                    )

            for b in range(B):
                xb = xpool.tile([8, PPB], f16, tag="xb")
                nc.sync.dma_start(xb[:], xrows_d[b])
                # S rows per batch: col-block r of 64 = [S0h,S1h,S2h,S0l,S1l,S2l]
                sd_all = sdp.tile([128, 6 * 64], f32, tag="sd")
                for Tc in range(NPT // 2):
                    sacc = psac.tile([128, 512], f32, tag="sacc")
                    for k in range(NK):
                        c = b * NK + k
                        r2t = psr2.tile([128, 1024], f32, tag="r2")
                        for t in range(2):
                            T = 2 * Tc + t
                            nc.tensor.matmul(
                                r2t[:, 512 * t : 512 * (t + 1)],
                                wr2s[:, 128 * c : 128 * (c + 1)],
                                xb[:, 512 * T : 512 * (T + 1)],
                                start=True,
                                stop=True,
                            )
                        lt = ltp.tile([128, 1024], f32, tag="lt")
                        nc.scalar.activation(
                            lt[:], r2t[:], AF.Ln, bias=params[:, c : c + 1]
                        )
                        wt = wtp.tile([128, 1024], f32, tag="wt")
                        nc.vector.scalar_tensor_tensor(
                            wt[:], lt[:], params[:, 8 + c : 9 + c], r2t[:],
                            ALU.mult, ALU.add,
                        )
                        g = gp.tile([128, 1024], f16, tag="g")
                        nc.scalar.activation(
                            g[:], wt[:], AF.Exp,
                            bias=params[:, 24 + c : 25 + c],
                            scale=params[:, 16 + c : 17 + c],
                        )
                        for t in range(2):
                            nc.tensor.matmul(
                                sacc[32 * t : 32 * t + 6, :],
                                wgs[:, 6 * c : 6 * (c + 1)],
                                g[:, 512 * t : 512 * (t + 1)],
                                start=(k == 0),
                                stop=(k == NK - 1),
                            )
                    stage = stp.tile([128, 512], f32, tag="stage")
                    nc.vector.tensor_copy(stage[0:38, :], sacc[0:38, :])
                    for t in range(2):
                        for r in range(6):
                            nc.sync.dma_start(
                                sd_all[
                                    16 * Tc + 8 * t : 16 * Tc + 8 * t + 8,
                                    64 * r : 64 * (r + 1),
                                ],
                                stage[32 * t + r : 32 * t + r + 1, :],
                            )
                # S_j = S_jh + S_jl, then u = px*S0 - S1, v = S2 - py*S0
                pyb = pf[:, (b * 2 + 0) * 64 : (b * 2 + 1) * 64]
                pxb = pf[:, (b * 2 + 1) * 64 : (b * 2 + 2) * 64]
                s0 = finp.tile([128, 64], f32, tag="s0")
                nc.vector.tensor_add(s0[:], sd_all[:, 0:64], sd_all[:, 192:256])
                s1 = finp.tile([128, 64], f32, tag="s1")
                nc.vector.tensor_add(s1[:], sd_all[:, 64:128], sd_all[:, 256:320])
                s2 = finp.tile([128, 64], f32, tag="s2")
                nc.vector.tensor_add(s2[:], sd_all[:, 128:192], sd_all[:, 320:384])
                tu = finp.tile([128, 64], f32, tag="tu")
                nc.vector.tensor_mul(tu[:], pxb, s0[:])
                u = finp.tile([128, 64], f32, tag="u")
                nc.vector.tensor_sub(u[:], tu[:], s1[:])
                tv = finp.tile([128, 64], f32, tag="tv")
                nc.vector.tensor_mul(tv[:], pyb, s0[:])
                v = finp.tile([128, 64], f32, tag="v")
                nc.vector.tensor_sub(v[:], s2[:], tv[:])
                nc.sync.dma_start(out_d[b, 0], u[:])
                nc.sync.dma_start(out_d[b, 1], v[:])
    nc.compile()
    return nc


def _hl(a):
    """fp16 hi/lo split of an fp32 array."""
    h = a.astype(np.float16)
    l = (a - h.astype(np.float32)).astype(np.float16)
    return h, l


def _prep_inputs(vortex_feature, points):
    vf = np.asarray(vortex_feature, dtype=np.float32)
    pts_full = np.asarray(points, dtype=np.float32)

    y = vf[:, :, 0]
    x = vf[:, :, 1]
    tau = vf[:, :, 2]
    sig = vf[:, :, 3]
    sig2 = sig * sig
    vv = y * y + x * x + EPS
    nisg = -1.0 / sig2
    chalf = 0.5 * sig2

    def blk(a):  # [B, N] -> [128, B*NK] with col = b*NK+k
        return np.ascontiguousarray(
            a.reshape(B, NK, 128).transpose(2, 0, 1).reshape(128, B * NK)
        )

    params = np.zeros((128, 32), dtype=np.float32)
    params[:, 0:8] = blk(vv)
    params[:, 8:16] = blk(chalf)
    params[:, 16:24] = blk(nisg)
    params[:, 24:32] = blk(nisg * vv)

    wyh, wyl = _hl(-2.0 * y)
    wxh, wxl = _hl(-2.0 * x)
    ones = np.ones_like(wyh)
    # row r of wr2 pairs with row r of xrows; big terms first so PSUM partial
    # sums cancel early (less fp32 accumulation error on near pairs)
    wstack = np.stack([ones, wyh, wxh, ones, wyh, wyl, wxh, wxl], axis=0)  # [8,B,N]
    wr2 = np.ascontiguousarray(
        wstack.reshape(8, B, NK, 128).reshape(8, B * NK * 128).astype(np.float16)
    )

    w0, w1, w2 = tau, tau * x, tau * y
    w0h, w0l = _hl(w0)
    w1h, w1l = _hl(w1)
    w2h, w2l = _hl(w2)
    wgf = np.stack([w0h, w1h, w2h, w0l, w1l, w2l], axis=-1)  # [B, N, 6]
    wg = np.ascontiguousarray(
        wgf.reshape(B, NK, 128, 6).transpose(2, 0, 1, 3).reshape(128, B * NK * 6)
    ).astype(np.float16)

    in_maps = []
    for i in range(NCORES):
        sl = pts_full[:, i * HPC : (i + 1) * HPC]          # [B, 32, 256, 2]
        flat = sl.reshape(B, PPB, 2)
        py = flat[:, :, 0]
        px = flat[:, :, 1]
        ph, pl = _hl(py)
        qh, ql = _hl(px)
        pp = py * py + px * px
        pph, ppl = _hl(pp)
        xrows = np.ascontiguousarray(
            np.stack([pph, ph, qh, ppl, pl, ph, ql, qh], axis=1)  # [B, 8, PPB]
        ).astype(np.float16)
        pts = np.ascontiguousarray(
            flat.transpose(0, 2, 1).reshape(B, 2, 128, PPB // 128)
        )
        in_maps.append(
            {"params": params, "wr2": wr2, "wg": wg, "xrows": xrows, "ptsf": pts}
        )
    return in_maps


def _assemble(results):
    out = np.zeros((B, H, W, 2), dtype=np.float32)
    for i in range(NCORES):
        o = np.asarray(results[i]["out"])  # [B, 2, 128, 64]
        o = o.reshape(B, 2, PPB).transpose(0, 2, 1).reshape(B, HPC, W, 2)
        out[:, i * HPC : (i + 1) * HPC] = o
    return out


def _run(vortex_feature, points, trace=False):
    _, _, _, run_bass_kernel_spmd, _b = _bass_modules()
    if "nc" not in _cache:
        _cache["nc"] = _build_nc()
    in_maps = _prep_inputs(vortex_feature, points)
    res = run_bass_kernel_spmd(
        _cache["nc"], in_maps, list(range(NCORES)), trace=trace
    )
    return _assemble(res.results), res


def kernel(vortex_feature, points):
    out, _ = _run(vortex_feature, points, trace=False)
    return out


# revision 6
# speedup vs baseline: 2.6571x; 2.0268x over previous
"""Gaussian falloff vortex-velocity kernel for Trainium2 (8 NeuronCores).

Math: out[b,h,w,:] = sum_n tau_n * exp(-r2/sig_n^2) / sqrt(r2) * (d2, -d1)
with d1 = py - y_n, d2 = px - x_n, r2 = d1^2 + d2^2.

Device-side structure (per core, H split 8 ways):
  1. r2 via TensorE:  r2 = pp - 2y*py - 2x*px + (y^2+x^2+eps), expanded as an
     8-row fp16 matmul (hi/lo split of each operand keeps fp32-level accuracy;
     products are exact in fp16xfp16->fp32, accumulation is fp32 PSUM).
     Output tile: [128 particles, 1024 points] across 2 PSUM banks.
  2. ACT:  lt = Ln(r2_mm + vv)         (bias = vv = y^2+x^2+eps, per-partition)
     DVE:  wt = chalf * lt + r2_mm     (chalf = 0.5*sig^2)
     ACT:  g  = Exp(nisg * wt + nisg*vv) -> fp16   (nisg = -1/sig^2)
     which equals exp(-r2/sig^2)/sqrt(r2).
  3. S-sums via TensorE: [6,512] = [tau,tau*x,tau*y] (hi/lo fp16) contracted
     over 128 particles, accumulated over the 4 particle blocks in PSUM,
     partition-stacked at offsets {0,32} for the two point-tiles of a chunk.
  4. u = px*S0 - S1, v = S2 - py*S0 on DVE.
"""

import sys

import numpy as np

B, H, W, N = 2, 256, 256, 512
NCORES = 8
HPC = H // NCORES          # 32 rows per core
PPB = HPC * W              # 8192 points per batch per core
NPT = PPB // 512           # 16 point-tiles of 512 per batch
NK = N // 128              # 4 particle blocks
EPS = 4e-6                 # keeps matmul-expanded r2 strictly positive

_cache = {}


def _bass_modules():
    if "/opt/trn_rl_repo" not in sys.path:
        sys.path.insert(0, "/opt/trn_rl_repo")
    import concourse.bass as bass
    import concourse.mybir as mybir
    import concourse.tile as tile
    from concourse import bacc
    from concourse.bass_utils import run_bass_kernel_spmd

    return bass, mybir, tile, run_bass_kernel_spmd, bacc


def _patch_act_tables(mybir, bacc):
    """Make Ln and Exp resolve to the combined natural_log_exp_and_others
    table set. The default first-match selection alternates between the
    natural_log and exp_and_others sets, inserting a ~1.3us ACT_TABLE_LOAD
    before nearly every activation (162us of pure table reloads per run)."""
    import concourse.hw_specs as hw_specs

    if getattr(bacc, "_act_tables_patched", False):
        return
    AF = mybir.ActivationFunctionType
    orig = hw_specs.get_activation_tables

    def patched(module_arch):
        tabs = orig(module_arch)
        out = {}
        for name, fns in tabs.items():
            if name != "natural_log_exp_and_others" and (
                AF.Ln in fns or AF.Exp in fns
            ):
                fns = fns - {AF.Ln, AF.Exp}
            out[name] = fns
        return out

    bacc.get_activation_tables = patched
    bacc._act_tables_patched = True


def _build_nc():
    bass, mybir, tile, _, bacc = _bass_modules()
    _patch_act_tables(mybir, bacc)
    f32 = mybir.dt.float32
    f16 = mybir.dt.float16
    AF = mybir.ActivationFunctionType
    ALU = mybir.AluOpType

    nc = bacc.Bacc(None)
    params_d = nc.declare_dram_parameter("params", [128, 32], f32, isOutput=False)
    wr2_d = nc.declare_dram_parameter("wr2", [8, B * NK * 128], f16, isOutput=False)
    wg_d = nc.declare_dram_parameter("wg", [128, B * NK * 6], f16, isOutput=False)
    xrows_d = nc.declare_dram_parameter("xrows", [B, 8, PPB], f16, isOutput=False)
    ptsf_d = nc.declare_dram_parameter("ptsf", [B, 2, 128, 64], f32, isOutput=False)
    out_d = nc.declare_dram_parameter("out", [B, 2, 128, 64], f32, isOutput=True)

    with tile.TileContext(nc) as tc:
        with (
            tc.tile_pool(name="const", bufs=1) as cpool,
            tc.tile_pool(name="xrows", bufs=2) as xpool,
            tc.tile_pool(name="lt", bufs=3) as ltp,
            tc.tile_pool(name="wt", bufs=3) as wtp,
            tc.tile_pool(name="g", bufs=3) as gp,
            tc.tile_pool(name="stage", bufs=2) as stp,
            tc.tile_pool(name="sd", bufs=2) as sdp,
            tc.tile_pool(name="fin", bufs=2) as finp,
            tc.tile_pool(name="psr2", bufs=2, space=bass.MemorySpace.PSUM) as psr2,
            tc.tile_pool(name="psac", bufs=2, space=bass.MemorySpace.PSUM) as psac,
        ):
            params = cpool.tile([128, 32], f32)
            nc.sync.dma_start(params[:], params_d[:])
            wr2s = cpool.tile([8, B * NK * 128], f16)
            nc.sync.dma_start(wr2s[:], wr2_d[:])
            wgs = cpool.tile([128, B * NK * 6], f16)
            nc.sync.dma_start(wgs[:], wg_d[:])
            pf = cpool.tile([128, B * 2 * 64], f32)
            for b in range(B):
                for a in range(2):
                    nc.sync.dma_start(
                        pf[:, (b * 2 + a) * 64 : (b * 2 + a + 1) * 64],
                        ptsf_d[b, a],
                    )

            for b in range(B):
                xb = xpool.tile([8, PPB], f16, tag="xb")
                nc.sync.dma_start(xb[:], xrows_d[b])
                # S rows per batch: col-block r of 64 = [S0h,S1h,S2h,S0l,S1l,S2l]
                sd_all = sdp.tile([128, 6 * 64], f32, tag="sd")
                for Tc in range(NPT // 2):
                    sacc = psac.tile([128, 512], f32, tag="sacc")
                    for k in range(NK):
                        c = b * NK + k
                        r2t = psr2.tile([128, 1024], f32, tag="r2")
                        for t in range(2):
                            T = 2 * Tc + t
                            nc.tensor.matmul(
                                r2t[:, 512 * t : 512 * (t + 1)],
                                wr2s[:, 128 * c : 128 * (c + 1)],
                                xb[:, 512 * T : 512 * (T + 1)],
                                start=True,
                                stop=True,
                            )
                        lt = ltp.tile([128, 1024], f32, tag="lt")
                        nc.scalar.activation(
                            lt[:], r2t[:], AF.Ln, bias=params[:, c : c + 1]
                        )
                        wt = wtp.tile([128, 1024], f32, tag="wt")
                        nc.vector.scalar_tensor_tensor(
                            wt[:], lt[:], params[:, 8 + c : 9 + c], r2t[:],
                            ALU.mult, ALU.add,
                        )
                        g = gp.tile([128, 1024], f16, tag="g")
                        nc.scalar.activation(
                            g[:], wt[:], AF.Exp,
                            bias=params[:, 24 + c : 25 + c],
                            scale=params[:, 16 + c : 17 + c],
                        )
                        for t in range(2):
                            nc.tensor.matmul(
                                sacc[32 * t : 32 * t + 6, :],
                                wgs[:, 6 * c : 6 * (c + 1)],
                                g[:, 512 * t : 512 * (t + 1)],
                                start=(k == 0),
                                stop=(k == NK - 1),
                            )
                    stage = stp.tile([128, 512], f32, tag="stage")
                    nc.vector.tensor_copy(stage[0:38, :], sacc[0:38, :])
                    for t in range(2):
                        for r in range(6):
                            nc.sync.dma_start(
                                sd_all[
                                    16 * Tc + 8 * t : 16 * Tc + 8 * t + 8,
                                    64 * r : 64 * (r + 1),
                                ],
                                stage[32 * t + r : 32 * t + r + 1, :],
                            )
                # S_j = S_jh + S_jl, then u = px*S0 - S1, v = S2 - py*S0
                pyb = pf[:, (b * 2 + 0) * 64 : (b * 2 + 1) * 64]
                pxb = pf[:, (b * 2 + 1) * 64 : (b * 2 + 2) * 64]
                s0 = finp.tile([128, 64], f32, tag="s0")
                nc.vector.tensor_add(s0[:], sd_all[:, 0:64], sd_all[:, 192:256])
                s1 = finp.tile([128, 64], f32, tag="s1")
                nc.vector.tensor_add(s1[:], sd_all[:, 64:128], sd_all[:, 256:320])
                s2 = finp.tile([128, 64], f32, tag="s2")
                nc.vector.tensor_add(s2[:], sd_all[:, 128:192], sd_all[:, 320:384])
                tu = finp.tile([128, 64], f32, tag="tu")
                nc.vector.tensor_mul(tu[:], pxb, s0[:])
                u = finp.tile([128, 64], f32, tag="u")
                nc.vector.tensor_sub(u[:], tu[:], s1[:])
                tv = finp.tile([128, 64], f32, tag="tv")
                nc.vector.tensor_mul(tv[:], pyb, s0[:])
                v = finp.tile([128, 64], f32, tag="v")
                nc.vector.tensor_sub(v[:], s2[:], tv[:])
                nc.sync.dma_start(out_d[b, 0], u[:])
                nc.sync.dma_start(out_d[b, 1], v[:])
    nc.compile()
    return nc


def _hl(a):
    """fp16 hi/lo split of an fp32 array."""
    h = a.astype(np.float16)
    l = (a - h.astype(np.float32)).astype(np.float16)
    return h, l


def _prep_inputs(vortex_feature, points):
    vf = np.asarray(vortex_feature, dtype=np.float32)
    pts_full = np.asarray(points, dtype=np.float32)

    y = vf[:, :, 0]
    x = vf[:, :, 1]
    tau = vf[:, :, 2]
    sig = vf[:, :, 3]
    sig2 = sig * sig
    vv = y * y + x * x + EPS
    nisg = -1.0 / sig2
    chalf = 0.5 * sig2

    def blk(a):  # [B, N] -> [128, B*NK] with col = b*NK+k
        return np.ascontiguousarray(
            a.reshape(B, NK, 128).transpose(2, 0, 1).reshape(128, B * NK)
        )

    params = np.zeros((128, 32), dtype=np.float32)
    params[:, 0:8] = blk(vv)
    params[:, 8:16] = blk(chalf)
    params[:, 16:24] = blk(nisg)
    params[:, 24:32] = blk(nisg * vv)

    wyh, wyl = _hl(-2.0 * y)
    wxh, wxl = _hl(-2.0 * x)
    ones = np.ones_like(wyh)
    # row r of wr2 pairs with row r of xrows; big terms first so PSUM partial
    # sums cancel early (less fp32 accumulation error on near pairs)
    wstack = np.stack([ones, wyh, wxh, ones, wyh, wyl, wxh, wxl], axis=0)  # [8,B,N]
    wr2 = np.ascontiguousarray(
        wstack.reshape(8, B, NK, 128).reshape(8, B * NK * 128).astype(np.float16)
    )

    w0, w1, w2 = tau, tau * x, tau * y
    w0h, w0l = _hl(w0)
    w1h, w1l = _hl(w1)
    w2h, w2l = _hl(w2)
    wgf = np.stack([w0h, w1h, w2h, w0l, w1l, w2l], axis=-1)  # [B, N, 6]
    wg = np.ascontiguousarray(
        wgf.reshape(B, NK, 128, 6).transpose(2, 0, 1, 3).reshape(128, B * NK * 6)
    ).astype(np.float16)

    in_maps = []
    for i in range(NCORES):
        sl = pts_full[:, i * HPC : (i + 1) * HPC]          # [B, 32, 256, 2]
        flat = sl.reshape(B, PPB, 2)
        py = flat[:, :, 0]
        px = flat[:, :, 1]
        ph, pl = _hl(py)
        qh, ql = _hl(px)
        pp = py * py + px * px
        pph, ppl = _hl(pp)
        xrows = np.ascontiguousarray(
            np.stack([pph, ph, qh, ppl, pl, ph, ql, qh], axis=1)  # [B, 8, PPB]
        ).astype(np.float16)
        pts = np.ascontiguousarray(
            flat.transpose(0, 2, 1).reshape(B, 2, 128, PPB // 128)
        )
        in_maps.append(
            {"params": params, "wr2": wr2, "wg": wg, "xrows": xrows, "ptsf": pts}
        )
    return in_maps


def _assemble(results):
    out = np.zeros((B, H, W, 2), dtype=np.float32)
    for i in range(NCORES):
        o = np.asarray(results[i]["out"])  # [B, 2, 128, 64]
        o = o.reshape(B, 2, PPB).transpose(0, 2, 1).reshape(B, HPC, W, 2)
        out[:, i * HPC : (i + 1) * HPC] = o
    return out


def _run(vortex_feature, points, trace=False):
    _, _, _, run_bass_kernel_spmd, _b = _bass_modules()
    if "nc" not in _cache:
        _cache["nc"] = _build_nc()
    in_maps = _prep_inputs(vortex_feature, points)
    res = run_bass_kernel_spmd(
        _cache["nc"], in_maps, list(range(NCORES)), trace=trace
    )
    return _assemble(res.results), res


def kernel(vortex_feature, points):
    out, _ = _run(vortex_feature, points, trace=False)
    return out


# revision 10
# speedup vs baseline: 2.7102x; 1.0200x over previous
"""Gaussian falloff vortex-velocity kernel for Trainium2 (8 NeuronCores).

Math: out[b,h,w,:] = sum_n tau_n * exp(-r2/sig_n^2) / sqrt(r2) * (d2, -d1)
with d1 = py - y_n, d2 = px - x_n, r2 = d1^2 + d2^2.

Device-side structure (per core, H split 8 ways):
  1. r2 via TensorE:  r2 = pp - 2y*py - 2x*px + (y^2+x^2+eps), expanded as an
     8-row fp16 matmul (hi/lo split of each operand keeps fp32-level accuracy;
     products are exact in fp16xfp16->fp32, accumulation is fp32 PSUM).
     Output tile: [128 particles, 1024 points] across 2 PSUM banks.
  2. ACT:  lt = Ln(r2_mm + vv)         (bias = vv = y^2+x^2+eps, per-partition)
     DVE:  wt = chalf * lt + r2_mm     (chalf = 0.5*sig^2)
     ACT:  g  = Exp(nisg * wt + nisg*vv) -> fp16   (nisg = -1/sig^2)
     which equals exp(-r2/sig^2)/sqrt(r2).
  3. S-sums via TensorE: [6,512] = [tau,tau*x,tau*y] (hi/lo fp16) contracted
     over 128 particles, accumulated over the 4 particle blocks in PSUM,
     partition-stacked at offsets {0,32} for the two point-tiles of a chunk.
  4. u = px*S0 - S1, v = S2 - py*S0 on DVE.
"""

import sys

import numpy as np

B, H, W, N = 2, 256, 256, 512
NCORES = 8
HPC = H // NCORES          # 32 rows per core
PPB = HPC * W              # 8192 points per batch per core
NPT = PPB // 512           # 16 point-tiles of 512 per batch
NK = N // 128              # 4 particle blocks
EPS = 4e-6                 # keeps matmul-expanded r2 strictly positive

_cache = {}


def _bass_modules():
    if "/opt/trn_rl_repo" not in sys.path:
        sys.path.insert(0, "/opt/trn_rl_repo")
    import concourse.bass as bass
    import concourse.mybir as mybir
    import concourse.tile as tile
    from concourse import bacc
    from concourse.bass_utils import run_bass_kernel_spmd

    return bass, mybir, tile, run_bass_kernel_spmd, bacc


def _patch_act_tables(mybir, bacc):
    """Make Ln and Exp resolve to the combined natural_log_exp_and_others
    table set. The default first-match selection alternates between the
    natural_log and exp_and_others sets, inserting a ~1.3us ACT_TABLE_LOAD
    before nearly every activation (162us of pure table reloads per run)."""
    import concourse.hw_specs as hw_specs

    if getattr(bacc, "_act_tables_patched", False):
        return
    AF = mybir.ActivationFunctionType
    orig = hw_specs.get_activation_tables

    def patched(module_arch):
        tabs = orig(module_arch)
        out = {}
        for name, fns in tabs.items():
            if name != "natural_log_exp_and_others" and (
                AF.Ln in fns or AF.Exp in fns
            ):
                fns = fns - {AF.Ln, AF.Exp}
            out[name] = fns
        return out

    bacc.get_activation_tables = patched
    bacc._act_tables_patched = True


def _build_nc():
    bass, mybir, tile, _, bacc = _bass_modules()
    _patch_act_tables(mybir, bacc)
    f32 = mybir.dt.float32
    f16 = mybir.dt.float16
    AF = mybir.ActivationFunctionType
    ALU = mybir.AluOpType

    nc = bacc.Bacc(None)
    params_d = nc.declare_dram_parameter("params", [128, 32], f32, isOutput=False)
    wr2_d = nc.declare_dram_parameter("wr2", [8, B * NK * 128], f16, isOutput=False)
    wg_d = nc.declare_dram_parameter("wg", [128, B * NK * 6], f16, isOutput=False)
    xrows_d = nc.declare_dram_parameter("xrows", [B, 8, PPB], f16, isOutput=False)
    ptsf_d = nc.declare_dram_parameter("ptsf", [B, 2, 128, 64], f32, isOutput=False)
    out_d = nc.declare_dram_parameter("out", [B, 2, 128, 64], f32, isOutput=True)

    with tile.TileContext(nc) as tc:
        with (
            tc.tile_pool(name="const", bufs=1) as cpool,
            tc.tile_pool(name="xrows", bufs=2) as xpool,
            tc.tile_pool(name="lt", bufs=3) as ltp,
            tc.tile_pool(name="wt", bufs=3) as wtp,
            tc.tile_pool(name="g", bufs=3) as gp,
            tc.tile_pool(name="stage", bufs=2) as stp,
            tc.tile_pool(name="sd", bufs=2) as sdp,
            tc.tile_pool(name="fin", bufs=2) as finp,
            tc.tile_pool(name="psr2", bufs=2, space=bass.MemorySpace.PSUM) as psr2,
            tc.tile_pool(name="psac", bufs=2, space=bass.MemorySpace.PSUM) as psac,
        ):
            wr2s = cpool.tile([8, B * NK * 128], f16)
            nc.sync.dma_start(wr2s[:], wr2_d[:])
            params = cpool.tile([128, 32], f32)
            nc.sync.dma_start(params[:], params_d[:])
            wgs = cpool.tile([128, B * NK * 6], f16)
            nc.sync.dma_start(wgs[:], wg_d[:])
            pf = cpool.tile([128, B * 2 * 64], f32)
            for b in range(B):
                for a in range(2):
                    nc.sync.dma_start(
                        pf[:, (b * 2 + a) * 64 : (b * 2 + a + 1) * 64],
                        ptsf_d[b, a],
                    )

            # chunks of point-tiles; 3 tiles = [128,1536] ACT ops, PSUM fits
            # psr2 (3 banks x 2 bufs) + psac (1 bank x 2 bufs) = 8 banks
            chunks = [(0, 3), (3, 3), (6, 3), (9, 3), (12, 3), (15, 1)]
            for b in range(B):
                xb = xpool.tile([8, PPB], f16, tag="xb")
                # first slice separately so chunk 0 can start ASAP
                nc.sync.dma_start(xb[:, 0:1536], xrows_d[b, :, 0:1536])
                nc.sync.dma_start(xb[:, 1536:PPB], xrows_d[b, :, 1536:PPB])
                # S rows per batch: col-block r of 64 = [S0h,S1h,S2h,S0l,S1l,S2l]
                sd_all = sdp.tile([128, 6 * 64], f32, tag="sd")
                for T0, ntil in chunks:
                    fd = 512 * ntil
                    sacc = psac.tile([128, 512], f32, tag="sacc")
                    for k in range(NK):
                        c = b * NK + k
                        r2t = psr2.tile([128, 1536], f32, tag="r2")
                        for t in range(ntil):
                            T = T0 + t
                            nc.tensor.matmul(
                                r2t[:, 512 * t : 512 * (t + 1)],
                                wr2s[:, 128 * c : 128 * (c + 1)],
                                xb[:, 512 * T : 512 * (T + 1)],
                                start=True,
                                stop=True,
                            )
                        lt = ltp.tile([128, 1536], f32, tag="lt")
                        nc.scalar.activation(
                            lt[:, 0:fd], r2t[:, 0:fd], AF.Ln,
                            bias=params[:, c : c + 1],
                        )
                        wt = wtp.tile([128, 1536], f32, tag="wt")
                        nc.vector.scalar_tensor_tensor(
                            wt[:, 0:fd], lt[:, 0:fd], params[:, 8 + c : 9 + c],
                            r2t[:, 0:fd], ALU.mult, ALU.add,
                        )
                        g = gp.tile([128, 1536], f16, tag="g")
                        nc.scalar.activation(
                            g[:, 0:fd], wt[:, 0:fd], AF.Exp,
                            bias=params[:, 24 + c : 25 + c],
                            scale=params[:, 16 + c : 17 + c],
                        )
                        for t in range(ntil):
                            nc.tensor.matmul(
                                sacc[32 * t : 32 * t + 6, :],
                                wgs[:, 6 * c : 6 * (c + 1)],
                                g[:, 512 * t : 512 * (t + 1)],
                                start=(k == 0),
                                stop=(k == NK - 1),
                            )
                    stage = stp.tile([128, 512], f32, tag="stage")
                    nc.vector.tensor_copy(
                        stage[0 : 32 * (ntil - 1) + 6, :],
                        sacc[0 : 32 * (ntil - 1) + 6, :],
                    )
                    # scatter [6 S-rows x 512] -> point-major, per tile
                    for t in range(ntil):
                        T = T0 + t
                        for r in range(6):
                            nc.sync.dma_start(
                                sd_all[8 * T : 8 * (T + 1), 64 * r : 64 * (r + 1)],
                                stage[32 * t + r : 32 * t + r + 1, :],
                            )
                # S_j = S_jh + S_jl, then u = px*S0 - S1, v = S2 - py*S0
                pyb = pf[:, (b * 2 + 0) * 64 : (b * 2 + 1) * 64]
                pxb = pf[:, (b * 2 + 1) * 64 : (b * 2 + 2) * 64]
                s0 = finp.tile([128, 64], f32, tag="s0")
                nc.vector.tensor_add(s0[:], sd_all[:, 0:64], sd_all[:, 192:256])
                s1 = finp.tile([128, 64], f32, tag="s1")
                nc.vector.tensor_add(s1[:], sd_all[:, 64:128], sd_all[:, 256:320])
                s2 = finp.tile([128, 64], f32, tag="s2")
                nc.vector.tensor_add(s2[:], sd_all[:, 128:192], sd_all[:, 320:384])
                tu = finp.tile([128, 64], f32, tag="tu")
                nc.vector.tensor_mul(tu[:], pxb, s0[:])
                u = finp.tile([128, 64], f32, tag="u")
                nc.vector.tensor_sub(u[:], tu[:], s1[:])
                tv = finp.tile([128, 64], f32, tag="tv")
                nc.vector.tensor_mul(tv[:], pyb, s0[:])
                v = finp.tile([128, 64], f32, tag="v")
                nc.vector.tensor_sub(v[:], s2[:], tv[:])
                nc.sync.dma_start(out_d[b, 0], u[:])
                nc.sync.dma_start(out_d[b, 1], v[:])
    nc.compile()
    return nc


def _hl(a):
    """fp16 hi/lo split of an fp32 array."""
    h = a.astype(np.float16)
    l = (a - h.astype(np.float32)).astype(np.float16)
    return h, l


def _prep_inputs(vortex_feature, points):
    vf = np.asarray(vortex_feature, dtype=np.float32)
    pts_full = np.asarray(points, dtype=np.float32)

    y = vf[:, :, 0]
    x = vf[:, :, 1]
    tau = vf[:, :, 2]
    sig = vf[:, :, 3]
    sig2 = sig * sig
    vv = y * y + x * x + EPS
    nisg = -1.0 / sig2
    chalf = 0.5 * sig2

    def blk(a):  # [B, N] -> [128, B*NK] with col = b*NK+k
        return np.ascontiguousarray(
            a.reshape(B, NK, 128).transpose(2, 0, 1).reshape(128, B * NK)
        )

    params = np.zeros((128, 32), dtype=np.float32)
    params[:, 0:8] = blk(vv)
    params[:, 8:16] = blk(chalf)
    params[:, 16:24] = blk(nisg)
    params[:, 24:32] = blk(nisg * vv)

    wyh, wyl = _hl(-2.0 * y)
    wxh, wxl = _hl(-2.0 * x)
    ones = np.ones_like(wyh)
    # row r of wr2 pairs with row r of xrows; big terms first so PSUM partial
    # sums cancel early (less fp32 accumulation error on near pairs)
    wstack = np.stack([ones, wyh, wxh, ones, wyh, wyl, wxh, wxl], axis=0)  # [8,B,N]
    wr2 = np.ascontiguousarray(
        wstack.reshape(8, B, NK, 128).reshape(8, B * NK * 128).astype(np.float16)
    )

    w0, w1, w2 = tau, tau * x, tau * y
    w0h, w0l = _hl(w0)
    w1h, w1l = _hl(w1)
    w2h, w2l = _hl(w2)
    wgf = np.stack([w0h, w1h, w2h, w0l, w1l, w2l], axis=-1)  # [B, N, 6]
    wg = np.ascontiguousarray(
        wgf.reshape(B, NK, 128, 6).transpose(2, 0, 1, 3).reshape(128, B * NK * 6)
    ).astype(np.float16)

    in_maps = []
    for i in range(NCORES):
        sl = pts_full[:, i * HPC : (i + 1) * HPC]          # [B, 32, 256, 2]
        flat = sl.reshape(B, PPB, 2)
        py = flat[:, :, 0]
        px = flat[:, :, 1]
        ph, pl = _hl(py)
        qh, ql = _hl(px)
        pp = py * py + px * px
        pph, ppl = _hl(pp)
        xrows = np.ascontiguousarray(
            np.stack([pph, ph, qh, ppl, pl, ph, ql, qh], axis=1)  # [B, 8, PPB]
        ).astype(np.float16)
        pts = np.ascontiguousarray(
            flat.transpose(0, 2, 1).reshape(B, 2, 128, PPB // 128)
        )
        in_maps.append(
            {"params": params, "wr2": wr2, "wg": wg, "xrows": xrows, "ptsf": pts}
        )
    return in_maps


def _assemble(results):
    out = np.zeros((B, H, W, 2), dtype=np.float32)
    for i in range(NCORES):
        o = np.asarray(results[i]["out"])  # [B, 2, 128, 64]
        o = o.reshape(B, 2, PPB).transpose(0, 2, 1).reshape(B, HPC, W, 2)
        out[:, i * HPC : (i + 1) * HPC] = o
    return out


def _run(vortex_feature, points, trace=False):
    _, _, _, run_bass_kernel_spmd, _b = _bass_modules()
    if "nc" not in _cache:
        _cache["nc"] = _build_nc()
    in_maps = _prep_inputs(vortex_feature, points)
    res = run_bass_kernel_spmd(
        _cache["nc"], in_maps, list(range(NCORES)), trace=trace
    )
    return _assemble(res.results), res


def kernel(vortex_feature, points):
    out, _ = _run(vortex_feature, points, trace=False)
    return out


# revision 17
# speedup vs baseline: 2.7531x; 1.0158x over previous
"""Gaussian falloff vortex-velocity kernel for Trainium2 (8 NeuronCores).

Math: out[b,h,w,:] = sum_n tau_n * exp(-r2/sig_n^2) / sqrt(r2) * (d2, -d1)
with d1 = py - y_n, d2 = px - x_n, r2 = d1^2 + d2^2.

Device-side structure (per core, H split 8 ways):
  1. r2 via TensorE:  r2 = pp - 2y*py - 2x*px + (y^2+x^2+eps), expanded as an
     8-row fp16 matmul (hi/lo split of each operand keeps fp32-level accuracy;
     products are exact in fp16xfp16->fp32, accumulation is fp32 PSUM).
     Output tile: [128 particles, 1024 points] across 2 PSUM banks.
  2. ACT:  lt = Ln(r2_mm + vv)         (bias = vv = y^2+x^2+eps, per-partition)
     DVE:  wt = chalf * lt + r2_mm     (chalf = 0.5*sig^2)
     ACT:  g  = Exp(nisg * wt + nisg*vv) -> fp16   (nisg = -1/sig^2)
     which equals exp(-r2/sig^2)/sqrt(r2).
  3. S-sums via TensorE: [6,512] = [tau,tau*x,tau*y] (hi/lo fp16) contracted
     over 128 particles, accumulated over the 4 particle blocks in PSUM,
     partition-stacked at offsets {0,32} for the two point-tiles of a chunk.
  4. u = px*S0 - S1, v = S2 - py*S0 on DVE.
"""

import sys

import numpy as np

B, H, W, N = 2, 256, 256, 512
NCORES = 8
HPC = H // NCORES          # 32 rows per core
PPB = HPC * W              # 8192 points per batch per core
NPT = PPB // 512           # 16 point-tiles of 512 per batch
NK = N // 128              # 4 particle blocks
EPS = 4e-6                 # keeps matmul-expanded r2 strictly positive

_cache = {}


def _bass_modules():
    if "/opt/trn_rl_repo" not in sys.path:
        sys.path.insert(0, "/opt/trn_rl_repo")
    import concourse.bass as bass
    import concourse.mybir as mybir
    import concourse.tile as tile
    from concourse import bacc
    from concourse.bass_utils import run_bass_kernel_spmd

    return bass, mybir, tile, run_bass_kernel_spmd, bacc


def _patch_act_tables(mybir, bacc):
    """Make Ln and Exp resolve to the combined natural_log_exp_and_others
    table set. The default first-match selection alternates between the
    natural_log and exp_and_others sets, inserting a ~1.3us ACT_TABLE_LOAD
    before nearly every activation (162us of pure table reloads per run)."""
    import concourse.hw_specs as hw_specs

    if getattr(bacc, "_act_tables_patched", False):
        return
    AF = mybir.ActivationFunctionType
    orig = hw_specs.get_activation_tables

    def patched(module_arch):
        tabs = orig(module_arch)
        out = {}
        for name, fns in tabs.items():
            if name != "natural_log_exp_and_others" and (
                AF.Ln in fns or AF.Exp in fns
            ):
                fns = fns - {AF.Ln, AF.Exp}
            out[name] = fns
        return out

    bacc.get_activation_tables = patched
    bacc._act_tables_patched = True


def _build_nc():
    bass, mybir, tile, _, bacc = _bass_modules()
    _patch_act_tables(mybir, bacc)
    f32 = mybir.dt.float32
    f16 = mybir.dt.float16
    AF = mybir.ActivationFunctionType
    ALU = mybir.AluOpType

    nc = bacc.Bacc(None)
    # params cols 0:32 = activation params, 32:@ = point coords (pf)
    params_d = nc.declare_dram_parameter(
        "params", [128, 32 + B * 2 * 64], f32, isOutput=False
    )
    wr2_d = nc.declare_dram_parameter("wr2", [8, B * NK * 128], f16, isOutput=False)
    wg_d = nc.declare_dram_parameter("wg", [128, B * NK * 6], f16, isOutput=False)
    xrows_d = nc.declare_dram_parameter("xrows", [B, 8, PPB], f16, isOutput=False)
    out_d = nc.declare_dram_parameter("out", [B, 2, 128, 64], f32, isOutput=True)

    with tile.TileContext(nc) as tc:
        with (
            tc.tile_pool(name="const", bufs=1) as cpool,
            tc.tile_pool(name="xrows", bufs=2) as xpool,
            tc.tile_pool(name="lt", bufs=3) as ltp,
            tc.tile_pool(name="wt", bufs=3) as wtp,
            tc.tile_pool(name="g", bufs=3) as gp,
            tc.tile_pool(name="stage", bufs=2) as stp,
            tc.tile_pool(name="sdrows", bufs=2) as srp,
            tc.tile_pool(name="sd", bufs=2) as sdp,
            tc.tile_pool(name="fin", bufs=2) as finp,
            tc.tile_pool(name="psr2", bufs=2, space=bass.MemorySpace.PSUM) as psr2,
            tc.tile_pool(name="psac", bufs=2, space=bass.MemorySpace.PSUM) as psac,
        ):
            wr2s = cpool.tile([8, B * NK * 128], f16)
            nc.sync.dma_start(wr2s[:], wr2_d[:])
            pall = cpool.tile([128, 32 + B * 2 * 64], f32)
            nc.sync.dma_start(pall[:], params_d[:])
            params = pall[:, 0:32]
            pf = pall[:, 32:]
            wgs = cpool.tile([128, B * NK * 6], f16)
            nc.sync.dma_start(wgs[:], wg_d[:])

            # chunks of point-tiles; 3 tiles = [128,1536] ACT ops, PSUM fits
            # psr2 (3 banks x 2 bufs) + psac (1 bank x 2 bufs) = 8 banks
            chunks = [(0, 3), (3, 3), (6, 3), (9, 3), (12, 3), (15, 1)]
            for b in range(B):
                xb = xpool.tile([8, PPB], f16, tag="xb")
                # first slice separately so chunk 0 can start ASAP
                nc.sync.dma_start(xb[:, 0:1536], xrows_d[b, :, 0:1536])
                nc.sync.dma_start(xb[:, 1536:PPB], xrows_d[b, :, 1536:PPB])
                # row-major S accumulator: row r = [S0h,S1h,S2h,S0l,S1l,S2l][r]
                sd_rows = srp.tile([6, PPB], f32, tag="sdr")
                # point-major S: col-block r of 64
                sd_all = sdp.tile([128, 6 * 64], f32, tag="sd")
                for T0, ntil in chunks:
                    fd = 512 * ntil
                    sacc = psac.tile([128, 512], f32, tag="sacc")
                    for k in range(NK):
                        c = b * NK + k
                        r2t = psr2.tile([128, 1536], f32, tag="r2")
                        for t in range(ntil):
                            T = T0 + t
                            nc.tensor.matmul(
                                r2t[:, 512 * t : 512 * (t + 1)],
                                wr2s[:, 128 * c : 128 * (c + 1)],
                                xb[:, 512 * T : 512 * (T + 1)],
                                start=True,
                                stop=True,
                            )
                        lt = ltp.tile([128, 1536], f32, tag="lt")
                        nc.scalar.activation(
                            lt[:, 0:fd], r2t[:, 0:fd], AF.Ln,
                            bias=params[:, c : c + 1],
                        )
                        wt = wtp.tile([128, 1536], f32, tag="wt")
                        nc.vector.scalar_tensor_tensor(
                            wt[:, 0:fd], lt[:, 0:fd], params[:, 8 + c : 9 + c],
                            r2t[:, 0:fd], ALU.mult, ALU.add,
                        )
                        g = gp.tile([128, 1536], f16, tag="g")
                        nc.scalar.activation(
                            g[:, 0:fd], wt[:, 0:fd], AF.Exp,
                            bias=params[:, 24 + c : 25 + c],
                            scale=params[:, 16 + c : 17 + c],
                        )
                        for t in range(ntil):
                            nc.tensor.matmul(
                                sacc[32 * t : 32 * t + 6, :],
                                wgs[:, 6 * c : 6 * (c + 1)],
                                g[:, 512 * t : 512 * (t + 1)],
                                start=(k == 0),
                                stop=(k == NK - 1),
                            )
                    stage = stp.tile([128, 512], f32, tag="stage")
                    nc.vector.tensor_copy(
                        stage[0 : 32 * (ntil - 1) + 6, :],
                        sacc[0 : 32 * (ntil - 1) + 6, :],
                    )
                    # straight copy into row-major accumulator, per tile
                    for t in range(ntil):
                        T = T0 + t
                        nc.sync.dma_start(
                            sd_rows[0:6, 512 * T : 512 * (T + 1)],
                            stage[32 * t : 32 * t + 6, :],
                        )
                # one scatter DMA per S-row: [1, 8192] -> [128, 64]
                for r in range(6):
                    nc.sync.dma_start(
                        sd_all[:, 64 * r : 64 * (r + 1)],
                        sd_rows[r : r + 1, :],
                    )
                # S_j = S_jh + S_jl, then u = px*S0 - S1, v = S2 - py*S0
                pyb = pf[:, (b * 2 + 0) * 64 : (b * 2 + 1) * 64]
                pxb = pf[:, (b * 2 + 1) * 64 : (b * 2 + 2) * 64]
                s0 = finp.tile([128, 64], f32, tag="s0")
                nc.vector.tensor_add(s0[:], sd_all[:, 0:64], sd_all[:, 192:256])
                s1 = finp.tile([128, 64], f32, tag="s1")
                nc.vector.tensor_add(s1[:], sd_all[:, 64:128], sd_all[:, 256:320])
                s2 = finp.tile([128, 64], f32, tag="s2")
                nc.vector.tensor_add(s2[:], sd_all[:, 128:192], sd_all[:, 320:384])
                tu = finp.tile([128, 64], f32, tag="tu")
                nc.vector.tensor_mul(tu[:], pxb, s0[:])
                u = finp.tile([128, 64], f32, tag="u")
                nc.vector.tensor_sub(u[:], tu[:], s1[:])
                tv = finp.tile([128, 64], f32, tag="tv")
                nc.vector.tensor_mul(tv[:], pyb, s0[:])
                v = finp.tile([128, 64], f32, tag="v")
                nc.vector.tensor_sub(v[:], s2[:], tv[:])
                nc.sync.dma_start(out_d[b, 0], u[:])
                nc.sync.dma_start(out_d[b, 1], v[:])
    nc.compile()
    return nc


def _hl(a):
    """fp16 hi/lo split of an fp32 array."""
    h = a.astype(np.float16)
    l = (a - h.astype(np.float32)).astype(np.float16)
    return h, l


def _prep_inputs(vortex_feature, points):
    vf = np.asarray(vortex_feature, dtype=np.float32)
    pts_full = np.asarray(points, dtype=np.float32)

    y = vf[:, :, 0]
    x = vf[:, :, 1]
    tau = vf[:, :, 2]
    sig = vf[:, :, 3]
    sig2 = sig * sig
    vv = y * y + x * x + EPS
    nisg = -1.0 / sig2
    chalf = 0.5 * sig2

    def blk(a):  # [B, N] -> [128, B*NK] with col = b*NK+k
        return np.ascontiguousarray(
            a.reshape(B, NK, 128).transpose(2, 0, 1).reshape(128, B * NK)
        )

    params = np.zeros((128, 32 + B * 2 * 64), dtype=np.float32)
    params[:, 0:8] = blk(vv)
    params[:, 8:16] = blk(chalf)
    params[:, 16:24] = blk(nisg)
    params[:, 24:32] = blk(nisg * vv)

    wyh, wyl = _hl(-2.0 * y)
    wxh, wxl = _hl(-2.0 * x)
    ones = np.ones_like(wyh)
    # row r of wr2 pairs with row r of xrows; big terms first so PSUM partial
    # sums cancel early (less fp32 accumulation error on near pairs)
    wstack = np.stack([ones, wyh, wxh, ones, wyh, wyl, wxh, wxl], axis=0)  # [8,B,N]
    wr2 = np.ascontiguousarray(
        wstack.reshape(8, B, NK, 128).reshape(8, B * NK * 128).astype(np.float16)
    )

    w0, w1, w2 = tau, tau * x, tau * y
    w0h, w0l = _hl(w0)
    w1h, w1l = _hl(w1)
    w2h, w2l = _hl(w2)
    wgf = np.stack([w0h, w1h, w2h, w0l, w1l, w2l], axis=-1)  # [B, N, 6]
    wg = np.ascontiguousarray(
        wgf.reshape(B, NK, 128, 6).transpose(2, 0, 1, 3).reshape(128, B * NK * 6)
    ).astype(np.float16)

    in_maps = []
    for i in range(NCORES):
        sl = pts_full[:, i * HPC : (i + 1) * HPC]          # [B, 32, 256, 2]
        flat = sl.reshape(B, PPB, 2)
        py = flat[:, :, 0]
        px = flat[:, :, 1]
        ph, pl = _hl(py)
        qh, ql = _hl(px)
        pp = py * py + px * px
        pph, ppl = _hl(pp)
        xrows = np.ascontiguousarray(
            np.stack([pph, ph, qh, ppl, pl, ph, ql, qh], axis=1)  # [B, 8, PPB]
        ).astype(np.float16)
        pts = flat.transpose(0, 2, 1).reshape(B, 2, 128, PPB // 128)
        pcore = params.copy()
        pcore[:, 32:] = pts.transpose(2, 0, 1, 3).reshape(128, B * 2 * 64)
        in_maps.append({"params": pcore, "wr2": wr2, "wg": wg, "xrows": xrows})
    return in_maps


def _assemble(results):
    out = np.zeros((B, H, W, 2), dtype=np.float32)
    for i in range(NCORES):
        o = np.asarray(results[i]["out"])  # [B, 2, 128, 64]
        o = o.reshape(B, 2, PPB).transpose(0, 2, 1).reshape(B, HPC, W, 2)
        out[:, i * HPC : (i + 1) * HPC] = o
    return out


def _run(vortex_feature, points, trace=False):
    _, _, _, run_bass_kernel_spmd, _b = _bass_modules()
    if "nc" not in _cache:
        _cache["nc"] = _build_nc()
    in_maps = _prep_inputs(vortex_feature, points)
    res = run_bass_kernel_spmd(
        _cache["nc"], in_maps, list(range(NCORES)), trace=trace
    )
    return _assemble(res.results), res


def kernel(vortex_feature, points):
    out, _ = _run(vortex_feature, points, trace=False)
    return out


# revision 25
# speedup vs baseline: 2.8838x; 1.0475x over previous
"""Gaussian falloff vortex-velocity kernel for Trainium2 (8 NeuronCores).

Math: out[b,h,w,:] = sum_n tau_n * exp(-r2/sig_n^2) / sqrt(r2) * (d2, -d1)
with d1 = py - y_n, d2 = px - x_n, r2 = d1^2 + d2^2.

Device-side structure (per core, H split 8 ways):
  1. r2 via TensorE:  r2 = pp - 2y*py - 2x*px + (y^2+x^2+eps), expanded as an
     8-row fp16 matmul (hi/lo split of each operand keeps fp32-level accuracy;
     products are exact in fp16xfp16->fp32, accumulation is fp32 PSUM).
     Output tile: [128 particles, 1024 points] across 2 PSUM banks.
  2. ACT:  lt = Ln(r2_mm + vv)         (bias = vv = y^2+x^2+eps, per-partition)
     DVE:  wt = chalf * lt + r2_mm     (chalf = 0.5*sig^2)
     ACT:  g  = Exp(nisg * wt + nisg*vv) -> fp16   (nisg = -1/sig^2)
     which equals exp(-r2/sig^2)/sqrt(r2).
  3. S-sums via TensorE: [6,512] = [tau,tau*x,tau*y] (hi/lo fp16) contracted
     over 128 particles, accumulated over the 4 particle blocks in PSUM,
     partition-stacked at offsets {0,32} for the two point-tiles of a chunk.
  4. u = px*S0 - S1, v = S2 - py*S0 on DVE.
"""

import sys

import numpy as np

B, H, W, N = 2, 256, 256, 512
NCORES = 8
HPC = H // NCORES          # 32 rows per core
PPB = HPC * W              # 8192 points per batch per core
NPT = PPB // 512           # 16 point-tiles of 512 per batch
NK = N // 128              # 4 particle blocks
EPS = 4e-6                 # keeps matmul-expanded r2 strictly positive

_cache = {}


def _bass_modules():
    if "/opt/trn_rl_repo" not in sys.path:
        sys.path.insert(0, "/opt/trn_rl_repo")
    import concourse.bass as bass
    import concourse.mybir as mybir
    import concourse.tile as tile
    from concourse import bacc
    from concourse.bass_utils import run_bass_kernel_spmd

    return bass, mybir, tile, run_bass_kernel_spmd, bacc


def _patch_act_tables(mybir, bacc):
    """Make Ln and Exp resolve to the combined natural_log_exp_and_others
    table set. The default first-match selection alternates between the
    natural_log and exp_and_others sets, inserting a ~1.3us ACT_TABLE_LOAD
    before nearly every activation (162us of pure table reloads per run)."""
    import concourse.hw_specs as hw_specs

    if getattr(bacc, "_act_tables_patched", False):
        return
    AF = mybir.ActivationFunctionType
    orig = hw_specs.get_activation_tables

    def patched(module_arch):
        tabs = orig(module_arch)
        out = {}
        for name, fns in tabs.items():
            if name != "natural_log_exp_and_others" and (
                AF.Ln in fns or AF.Exp in fns
            ):
                fns = fns - {AF.Ln, AF.Exp}
            out[name] = fns
        return out

    bacc.get_activation_tables = patched
    bacc._act_tables_patched = True


def _build_nc():
    bass, mybir, tile, _, bacc = _bass_modules()
    _patch_act_tables(mybir, bacc)
    f32 = mybir.dt.float32
    f16 = mybir.dt.float16
    AF = mybir.ActivationFunctionType
    ALU = mybir.AluOpType

    nc = bacc.Bacc(None)
    # params cols 0:32 = activation params, 32: = [px | -py] point-major per b
    params_d = nc.declare_dram_parameter(
        "params", [128, 32 + B * 128], f32, isOutput=False
    )
    # w16: rows 0:8, cols 0:1024 = r2-matmul weights; cols 1024:1088 = S weights
    w16_d = nc.declare_dram_parameter(
        "w16", [128, B * NK * 128 + B * NK * 8], f16, isOutput=False
    )
    xrows_d = nc.declare_dram_parameter("xrows", [B, 8, PPB], f16, isOutput=False)
    out_d = nc.declare_dram_parameter("out", [B, 2, 128, 64], f32, isOutput=True)

    with tile.TileContext(nc) as tc:
        with (
            tc.tile_pool(name="const", bufs=1) as cpool,
            tc.tile_pool(name="xrows", bufs=2) as xpool,
            tc.tile_pool(name="lt", bufs=3) as ltp,
            tc.tile_pool(name="wt", bufs=3) as wtp,
            tc.tile_pool(name="g", bufs=3) as gp,
            tc.tile_pool(name="stage", bufs=2) as stp,
            tc.tile_pool(name="sd", bufs=2) as sdp,
            tc.tile_pool(name="fin", bufs=2) as finp,
            tc.tile_pool(name="dscratch", bufs=2, space="DRAM") as dpool,
            tc.tile_pool(name="psr2", bufs=2, space=bass.MemorySpace.PSUM) as psr2,
            tc.tile_pool(name="psac", bufs=2, space=bass.MemorySpace.PSUM) as psac,
        ):
            w16 = cpool.tile([128, B * NK * 128 + B * NK * 8], f16)
            nc.sync.dma_start(w16[:], w16_d[:])
            pall = cpool.tile([128, 32 + B * 128], f32)
            nc.sync.dma_start(pall[:], params_d[:])
            params = pall[:, 0:32]
            WG0 = B * NK * 128

            # chunks of point-tiles; 3 tiles = [128,1536] ACT ops, PSUM fits
            # psr2 (3 banks x 2 bufs) + psac (1 bank x 2 bufs) = 8 banks
            chunks = [(0, 3), (3, 3), (6, 3), (9, 3), (12, 3), (15, 1)]
            for b in range(B):
                xb = xpool.tile([8, PPB], f16, tag="xb")
                # first slice separately so chunk 0 can start ASAP
                nc.sync.dma_start(xb[:, 0:1536], xrows_d[b, :, 0:1536])
                nc.sync.dma_start(xb[:, 1536:PPB], xrows_d[b, :, 1536:PPB])
                # DRAM bounce for S rows: [S0h,S0h,S1'h,S2h,S0l,S0l,S1'l,S2l]
                sc = dpool.tile([8, PPB], f32, tag="sc")
                for T0, ntil in chunks:
                    fd = 512 * ntil
                    sacc = psac.tile([128, 512], f32, tag="sacc")
                    for k in range(NK):
                        c = b * NK + k
                        r2t = psr2.tile([128, 1536], f32, tag="r2")
                        for t in range(ntil):
                            T = T0 + t
                            nc.tensor.matmul(
                                r2t[:, 512 * t : 512 * (t + 1)],
                                w16[0:8, 128 * c : 128 * (c + 1)],
                                xb[:, 512 * T : 512 * (T + 1)],
                                start=True,
                                stop=True,
                            )
                        lt = ltp.tile([128, 1536], f32, tag="lt")
                        nc.scalar.activation(
                            lt[:, 0:fd], r2t[:, 0:fd], AF.Ln,
                            bias=params[:, c : c + 1],
                        )
                        wt = wtp.tile([128, 1536], f32, tag="wt")
                        nc.vector.scalar_tensor_tensor(
                            wt[:, 0:fd], lt[:, 0:fd], params[:, 8 + c : 9 + c],
                            r2t[:, 0:fd], ALU.mult, ALU.add,
                        )
                        g = gp.tile([128, 1536], f16, tag="g")
                        nc.scalar.activation(
                            g[:, 0:fd], wt[:, 0:fd], AF.Exp,
                            bias=params[:, 24 + c : 25 + c],
                            scale=params[:, 16 + c : 17 + c],
                        )
                        for t in range(ntil):
                            nc.tensor.matmul(
                                sacc[32 * t : 32 * t + 8, :],
                                w16[:, WG0 + 8 * c : WG0 + 8 * (c + 1)],
                                g[:, 512 * t : 512 * (t + 1)],
                                start=(k == 0),
                                stop=(k == NK - 1),
                            )
                    stage = stp.tile([128, 512], f32, tag="stage")
                    nc.vector.tensor_copy(
                        stage[0 : 32 * (ntil - 1) + 8, :],
                        sacc[0 : 32 * (ntil - 1) + 8, :],
                    )
                    # straight copy to DRAM bounce rows, per tile
                    for t in range(ntil):
                        T = T0 + t
                        nc.sync.dma_start(
                            sc[:, 512 * T : 512 * (T + 1)],
                            stage[32 * t : 32 * t + 8, :],
                        )
                # single gather: sd_all[p, 64r+c] = sc[r, 64p+c]
                sd_all = sdp.tile([128, 8 * 64], f32, tag="sd")
                nc.sync.dma_start(
                    sd_all[:].rearrange("p (r c) -> p r c", c=64),
                    sc[:].rearrange("r (p c) -> p r c", c=64),
                )
                # Ssum = [S0|S0|S1'|S2], uv = [px|-py] * [S0|S0] + [S1'|S2]
                pf2 = pall[:, 32 + b * 128 : 32 + (b + 1) * 128]
                ssum = finp.tile([128, 256], f32, tag="ssum")
                nc.vector.tensor_add(ssum[:], sd_all[:, 0:256], sd_all[:, 256:512])
                m = finp.tile([128, 128], f32, tag="m")
                nc.vector.tensor_mul(m[:], pf2, ssum[:, 0:128])
                uv = finp.tile([128, 128], f32, tag="uv")
                nc.vector.tensor_add(uv[:], m[:], ssum[:, 128:256])
                nc.sync.dma_start(
                    out_d[b].rearrange("a p c -> p a c"),
                    uv[:].rearrange("p (a c) -> p a c", c=64),
                )
    nc.compile()
    return nc


def _hl(a):
    """fp16 hi/lo split of an fp32 array."""
    h = a.astype(np.float16)
    l = (a - h.astype(np.float32)).astype(np.float16)
    return h, l


def _prep_inputs(vortex_feature, points):
    vf = np.asarray(vortex_feature, dtype=np.float32)
    pts_full = np.asarray(points, dtype=np.float32)

    y = vf[:, :, 0]
    x = vf[:, :, 1]
    tau = vf[:, :, 2]
    sig = vf[:, :, 3]
    sig2 = sig * sig
    vv = y * y + x * x + EPS
    nisg = -1.0 / sig2
    chalf = 0.5 * sig2

    def blk(a):  # [B, N] -> [128, B*NK] with col = b*NK+k
        return np.ascontiguousarray(
            a.reshape(B, NK, 128).transpose(2, 0, 1).reshape(128, B * NK)
        )

    params = np.zeros((128, 32 + B * 128), dtype=np.float32)
    params[:, 0:8] = blk(vv)
    params[:, 8:16] = blk(chalf)
    params[:, 16:24] = blk(nisg)
    params[:, 24:32] = blk(nisg * vv)

    wyh, wyl = _hl(-2.0 * y)
    wxh, wxl = _hl(-2.0 * x)
    ones = np.ones_like(wyh)
    # row r of wr2 pairs with row r of xrows; big terms first so PSUM partial
    # sums cancel early (less fp32 accumulation error on near pairs)
    wstack = np.stack([ones, wyh, wxh, ones, wyh, wyl, wxh, wxl], axis=0)  # [8,B,N]
    wr2 = np.ascontiguousarray(
        wstack.reshape(8, B, NK, 128).reshape(8, B * NK * 128).astype(np.float16)
    )

    # S columns: [S0, S0, S1', S2] with S1' = sum(-tau*x*g), S2 = sum(tau*y*g)
    # so uv = [px|-py] * [S0|S0] + [S1'|S2] gives (u, v) directly
    w0, w1, w2 = tau, -tau * x, tau * y
    w0h, w0l = _hl(w0)
    w1h, w1l = _hl(w1)
    w2h, w2l = _hl(w2)
    wgf = np.stack([w0h, w0h, w1h, w2h, w0l, w0l, w1l, w2l], axis=-1)  # [B,N,8]
    wg = np.ascontiguousarray(
        wgf.reshape(B, NK, 128, 8).transpose(2, 0, 1, 3).reshape(128, B * NK * 8)
    ).astype(np.float16)
    w16 = np.zeros((128, B * NK * 128 + B * NK * 8), dtype=np.float16)
    w16[0:8, 0 : B * NK * 128] = wr2
    w16[:, B * NK * 128 :] = wg

    in_maps = []
    for i in range(NCORES):
        sl = pts_full[:, i * HPC : (i + 1) * HPC]          # [B, 32, 256, 2]
        flat = sl.reshape(B, PPB, 2)
        py = flat[:, :, 0]
        px = flat[:, :, 1]
        ph, pl = _hl(py)
        qh, ql = _hl(px)
        pp = py * py + px * px
        pph, ppl = _hl(pp)
        xrows = np.ascontiguousarray(
            np.stack([pph, ph, qh, ppl, pl, ph, ql, qh], axis=1)  # [B, 8, PPB]
        ).astype(np.float16)
        pts = flat.transpose(0, 2, 1).reshape(B, 2, 128, PPB // 128)
        pcore = params.copy()
        for b in range(B):
            pcore[:, 32 + b * 128 : 32 + b * 128 + 64] = pts[b, 1]   # px
            pcore[:, 32 + b * 128 + 64 : 32 + (b + 1) * 128] = -pts[b, 0]  # -py
        in_maps.append({"params": pcore, "w16": w16, "xrows": xrows})
    return in_maps


def _assemble(results):
    out = np.zeros((B, H, W, 2), dtype=np.float32)
    for i in range(NCORES):
        o = np.asarray(results[i]["out"])  # [B, 2, 128, 64]
        o = o.reshape(B, 2, PPB).transpose(0, 2, 1).reshape(B, HPC, W, 2)
        out[:, i * HPC : (i + 1) * HPC] = o
    return out


def _run(vortex_feature, points, trace=False):
    _, _, _, run_bass_kernel_spmd, _b = _bass_modules()
    if "nc" not in _cache:
        _cache["nc"] = _build_nc()
    in_maps = _prep_inputs(vortex_feature, points)
    res = run_bass_kernel_spmd(
        _cache["nc"], in_maps, list(range(NCORES)), trace=trace
    )
    return _assemble(res.results), res


def kernel(vortex_feature, points):
    out, _ = _run(vortex_feature, points, trace=False)
    return out


# revision 28
# speedup vs baseline: 2.8988x; 1.0052x over previous
"""Gaussian falloff vortex-velocity kernel for Trainium2 (8 NeuronCores).

Math: out[b,h,w,:] = sum_n tau_n * exp(-r2/sig_n^2) / sqrt(r2) * (d2, -d1)
with d1 = py - y_n, d2 = px - x_n, r2 = d1^2 + d2^2.

Device-side structure (per core, H split 8 ways):
  1. r2 via TensorE:  r2 = pp - 2y*py - 2x*px + (y^2+x^2+eps), expanded as an
     8-row fp16 matmul (hi/lo split of each operand keeps fp32-level accuracy;
     products are exact in fp16xfp16->fp32, accumulation is fp32 PSUM).
     Output tile: [128 particles, 1024 points] across 2 PSUM banks.
  2. ACT:  lt = Ln(r2_mm + vv)         (bias = vv = y^2+x^2+eps, per-partition)
     DVE:  wt = chalf * lt + r2_mm     (chalf = 0.5*sig^2)
     ACT:  g  = Exp(nisg * wt + nisg*vv) -> fp16   (nisg = -1/sig^2)
     which equals exp(-r2/sig^2)/sqrt(r2).
  3. S-sums via TensorE: [6,512] = [tau,tau*x,tau*y] (hi/lo fp16) contracted
     over 128 particles, accumulated over the 4 particle blocks in PSUM,
     partition-stacked at offsets {0,32} for the two point-tiles of a chunk.
  4. u = px*S0 - S1, v = S2 - py*S0 on DVE.
"""

import sys

import numpy as np

B, H, W, N = 2, 256, 256, 512
NCORES = 8
HPC = H // NCORES          # 32 rows per core
PPB = HPC * W              # 8192 points per batch per core
NPT = PPB // 512           # 16 point-tiles of 512 per batch
NK = N // 128              # 4 particle blocks
EPS = 4e-6                 # keeps matmul-expanded r2 strictly positive

_cache = {}


def _bass_modules():
    if "/opt/trn_rl_repo" not in sys.path:
        sys.path.insert(0, "/opt/trn_rl_repo")
    import concourse.bass as bass
    import concourse.mybir as mybir
    import concourse.tile as tile
    from concourse import bacc
    from concourse.bass_utils import run_bass_kernel_spmd

    return bass, mybir, tile, run_bass_kernel_spmd, bacc


def _patch_act_tables(mybir, bacc):
    """Make Ln and Exp resolve to the combined natural_log_exp_and_others
    table set. The default first-match selection alternates between the
    natural_log and exp_and_others sets, inserting a ~1.3us ACT_TABLE_LOAD
    before nearly every activation (162us of pure table reloads per run)."""
    import concourse.hw_specs as hw_specs

    if getattr(bacc, "_act_tables_patched", False):
        return
    AF = mybir.ActivationFunctionType
    orig = hw_specs.get_activation_tables

    def patched(module_arch):
        tabs = orig(module_arch)
        out = {}
        for name, fns in tabs.items():
            if name != "natural_log_exp_and_others" and (
                AF.Ln in fns or AF.Exp in fns
            ):
                fns = fns - {AF.Ln, AF.Exp}
            out[name] = fns
        return out

    bacc.get_activation_tables = patched
    bacc._act_tables_patched = True


def _build_nc():
    bass, mybir, tile, _, bacc = _bass_modules()
    _patch_act_tables(mybir, bacc)
    f32 = mybir.dt.float32
    f16 = mybir.dt.float16
    AF = mybir.ActivationFunctionType
    ALU = mybir.AluOpType

    nc = bacc.Bacc(None)
    # params cols 0:32 = activation params, 32: = [px | -py] point-major per b
    params_d = nc.declare_dram_parameter(
        "params", [128, 32 + B * 128], f32, isOutput=False
    )
    # w16: rows 0:8, cols 0:1024 = r2-matmul weights; cols 1024:1088 = S weights
    w16_d = nc.declare_dram_parameter(
        "w16", [128, B * NK * 128 + B * NK * 8], f16, isOutput=False
    )
    xrows_d = nc.declare_dram_parameter("xrows", [B, 8, PPB], f16, isOutput=False)
    out_d = nc.declare_dram_parameter("out", [B, 2, 128, 64], f32, isOutput=True)

    with tile.TileContext(nc) as tc:
        with (
            tc.tile_pool(name="const", bufs=1) as cpool,
            tc.tile_pool(name="xrows", bufs=2) as xpool,
            tc.tile_pool(name="lt", bufs=3) as ltp,
            tc.tile_pool(name="wt", bufs=3) as wtp,
            tc.tile_pool(name="g", bufs=3) as gp,
            tc.tile_pool(name="stage", bufs=2) as stp,
            tc.tile_pool(name="sd", bufs=2) as sdp,
            tc.tile_pool(name="fin", bufs=2) as finp,
            tc.tile_pool(name="dscratch", bufs=2, space="DRAM") as dpool,
            tc.tile_pool(name="psr2", bufs=2, space=bass.MemorySpace.PSUM) as psr2,
            tc.tile_pool(name="psac", bufs=2, space=bass.MemorySpace.PSUM) as psac,
        ):
            WG0 = B * NK * 128
            w16 = cpool.tile([128, WG0 + B * NK * 8], f16)
            # r2-matmul weights are on the critical path to the first matmul
            nc.sync.dma_start(w16[0:8, 0:WG0], w16_d[0:8, 0:WG0])
            pall = cpool.tile([128, 32 + B * 128], f32)
            params = pall[:, 0:32]

            # chunks of point-tiles; 3 tiles = [128,1536] ACT ops, PSUM fits
            # psr2 (3 banks x 2 bufs) + psac (1 bank x 2 bufs) = 8 banks
            chunks = [(0, 3), (3, 3), (6, 3), (9, 3), (12, 3), (15, 1)]
            for b in range(B):
                xb = xpool.tile([8, PPB], f16, tag="xb")
                # first slice separately so chunk 0 can start ASAP
                nc.sync.dma_start(xb[:, 0:1536], xrows_d[b, :, 0:1536])
                if b == 0:
                    nc.sync.dma_start(pall[:], params_d[:])
                nc.sync.dma_start(xb[:, 1536:PPB], xrows_d[b, :, 1536:PPB])
                if b == 0:
                    nc.sync.dma_start(w16[:, WG0:], w16_d[:, WG0:])
                # DRAM bounce for S rows: [S0h,S0h,S1'h,S2h,S0l,S0l,S1'l,S2l]
                sc = dpool.tile([8, PPB], f32, tag="sc")
                sd_all = sdp.tile([128, 8 * 64], f32, tag="sd")
                for T0, ntil in chunks:
                    fd = 512 * ntil
                    sacc = psac.tile([128, 512], f32, tag="sacc")
                    for k in range(NK):
                        c = b * NK + k
                        r2t = psr2.tile([128, 1536], f32, tag="r2")
                        for t in range(ntil):
                            T = T0 + t
                            nc.tensor.matmul(
                                r2t[:, 512 * t : 512 * (t + 1)],
                                w16[0:8, 128 * c : 128 * (c + 1)],
                                xb[:, 512 * T : 512 * (T + 1)],
                                start=True,
                                stop=True,
                            )
                        lt = ltp.tile([128, 1536], f32, tag="lt")
                        nc.scalar.activation(
                            lt[:, 0:fd], r2t[:, 0:fd], AF.Ln,
                            bias=params[:, c : c + 1],
                        )
                        wt = wtp.tile([128, 1536], f32, tag="wt")
                        nc.vector.scalar_tensor_tensor(
                            wt[:, 0:fd], lt[:, 0:fd], params[:, 8 + c : 9 + c],
                            r2t[:, 0:fd], ALU.mult, ALU.add,
                        )
                        g = gp.tile([128, 1536], f16, tag="g")
                        nc.scalar.activation(
                            g[:, 0:fd], wt[:, 0:fd], AF.Exp,
                            bias=params[:, 24 + c : 25 + c],
                            scale=params[:, 16 + c : 17 + c],
                        )
                        for t in range(ntil):
                            nc.tensor.matmul(
                                sacc[32 * t : 32 * t + 8, :],
                                w16[:, WG0 + 8 * c : WG0 + 8 * (c + 1)],
                                g[:, 512 * t : 512 * (t + 1)],
                                start=(k == 0),
                                stop=(k == NK - 1),
                            )
                    stage = stp.tile([128, 512], f32, tag="stage")
                    nc.vector.tensor_copy(
                        stage[0 : 32 * (ntil - 1) + 8, :],
                        sacc[0 : 32 * (ntil - 1) + 8, :],
                    )
                    # straight copy to DRAM bounce rows, per tile
                    for t in range(ntil):
                        T = T0 + t
                        nc.sync.dma_start(
                            sc[:, 512 * T : 512 * (T + 1)],
                            stage[32 * t : 32 * t + 8, :],
                        )
                    # per-chunk gather: sd_all[p, 64r+c] = sc[r, 64p+c]
                    with tc.high_priority():
                        nc.sync.dma_start(
                            sd_all[8 * T0 : 8 * (T0 + ntil), :].rearrange(
                                "p (r c) -> p r c", c=64
                            ),
                            sc[:, 512 * T0 : 512 * (T0 + ntil)].rearrange(
                                "r (p c) -> p r c", c=64
                            ),
                        )
                # Ssum = [S0|S0|S1'|S2], uv = [px|-py] * [S0|S0] + [S1'|S2]
                with tc.high_priority():
                    pf2 = pall[:, 32 + b * 128 : 32 + (b + 1) * 128]
                    ssum = finp.tile([128, 256], f32, tag="ssum")
                    nc.vector.tensor_add(
                        ssum[:], sd_all[:, 0:256], sd_all[:, 256:512]
                    )
                    m = finp.tile([128, 128], f32, tag="m")
                    nc.vector.tensor_mul(m[:], pf2, ssum[:, 0:128])
                    uv = finp.tile([128, 128], f32, tag="uv")
                    nc.vector.tensor_add(uv[:], m[:], ssum[:, 128:256])
                    nc.sync.dma_start(
                        out_d[b].rearrange("a p c -> p a c"),
                        uv[:].rearrange("p (a c) -> p a c", c=64),
                    )
    nc.compile()
    return nc


def _hl(a):
    """fp16 hi/lo split of an fp32 array."""
    h = a.astype(np.float16)
    l = (a - h.astype(np.float32)).astype(np.float16)
    return h, l


def _prep_inputs(vortex_feature, points):
    vf = np.asarray(vortex_feature, dtype=np.float32)
    pts_full = np.asarray(points, dtype=np.float32)

    y = vf[:, :, 0]
    x = vf[:, :, 1]
    tau = vf[:, :, 2]
    sig = vf[:, :, 3]
    sig2 = sig * sig
    vv = y * y + x * x + EPS
    nisg = -1.0 / sig2
    chalf = 0.5 * sig2

    def blk(a):  # [B, N] -> [128, B*NK] with col = b*NK+k
        return np.ascontiguousarray(
            a.reshape(B, NK, 128).transpose(2, 0, 1).reshape(128, B * NK)
        )

    params = np.zeros((128, 32 + B * 128), dtype=np.float32)
    params[:, 0:8] = blk(vv)
    params[:, 8:16] = blk(chalf)
    params[:, 16:24] = blk(nisg)
    params[:, 24:32] = blk(nisg * vv)

    wyh, wyl = _hl(-2.0 * y)
    wxh, wxl = _hl(-2.0 * x)
    ones = np.ones_like(wyh)
    # row r of wr2 pairs with row r of xrows; big terms first so PSUM partial
    # sums cancel early (less fp32 accumulation error on near pairs)
    wstack = np.stack([ones, wyh, wxh, ones, wyh, wyl, wxh, wxl], axis=0)  # [8,B,N]
    wr2 = np.ascontiguousarray(
        wstack.reshape(8, B, NK, 128).reshape(8, B * NK * 128).astype(np.float16)
    )

    # S columns: [S0, S0, S1', S2] with S1' = sum(-tau*x*g), S2 = sum(tau*y*g)
    # so uv = [px|-py] * [S0|S0] + [S1'|S2] gives (u, v) directly
    w0, w1, w2 = tau, -tau * x, tau * y
    w0h, w0l = _hl(w0)
    w1h, w1l = _hl(w1)
    w2h, w2l = _hl(w2)
    wgf = np.stack([w0h, w0h, w1h, w2h, w0l, w0l, w1l, w2l], axis=-1)  # [B,N,8]
    wg = np.ascontiguousarray(
        wgf.reshape(B, NK, 128, 8).transpose(2, 0, 1, 3).reshape(128, B * NK * 8)
    ).astype(np.float16)
    w16 = np.zeros((128, B * NK * 128 + B * NK * 8), dtype=np.float16)
    w16[0:8, 0 : B * NK * 128] = wr2
    w16[:, B * NK * 128 :] = wg

    in_maps = []
    for i in range(NCORES):
        sl = pts_full[:, i * HPC : (i + 1) * HPC]          # [B, 32, 256, 2]
        flat = sl.reshape(B, PPB, 2)
        py = flat[:, :, 0]
        px = flat[:, :, 1]
        ph, pl = _hl(py)
        qh, ql = _hl(px)
        pp = py * py + px * px
        pph, ppl = _hl(pp)
        xrows = np.ascontiguousarray(
            np.stack([pph, ph, qh, ppl, pl, ph, ql, qh], axis=1)  # [B, 8, PPB]
        ).astype(np.float16)
        pts = flat.transpose(0, 2, 1).reshape(B, 2, 128, PPB // 128)
        pcore = params.copy()
        for b in range(B):
            pcore[:, 32 + b * 128 : 32 + b * 128 + 64] = pts[b, 1]   # px
            pcore[:, 32 + b * 128 + 64 : 32 + (b + 1) * 128] = -pts[b, 0]  # -py
        in_maps.append({"params": pcore, "w16": w16, "xrows": xrows})
    return in_maps


def _assemble(results):
    out = np.zeros((B, H, W, 2), dtype=np.float32)
    for i in range(NCORES):
        o = np.asarray(results[i]["out"])  # [B, 2, 128, 64]
        o = o.reshape(B, 2, PPB).transpose(0, 2, 1).reshape(B, HPC, W, 2)
        out[:, i * HPC : (i + 1) * HPC] = o
    return out


def _run(vortex_feature, points, trace=False):
    _, _, _, run_bass_kernel_spmd, _b = _bass_modules()
    if "nc" not in _cache:
        _cache["nc"] = _build_nc()
    in_maps = _prep_inputs(vortex_feature, points)
    res = run_bass_kernel_spmd(
        _cache["nc"], in_maps, list(range(NCORES)), trace=trace
    )
    return _assemble(res.results), res


def kernel(vortex_feature, points):
    out, _ = _run(vortex_feature, points, trace=False)
    return out
